# revision 18
# baseline (speedup 1.0000x reference)
"""Bilinear interaction layer (pairwise per-field Linear + gate) on 8 trn2 cores.

out[b, p, :] = (femb[b, i_p] @ W[p].T) * femb[b, j_p]   for the P=C(F,2) field
pairs (i_p, j_p) in itertools.combinations order.  B=4096, F=30, D=128, P=435.

Sharding: data-parallel over batch (4096 -> 512 per core), W replicated.

Per core, pairs are processed in "i-blocks" (the (29-i) pairs sharing first
field i, consecutive in p).  For each i-block and each 128-row batch chunk,
TensorE runs fp32 matmuls with the v_i chunk [d=128, b=128] stationary and up
to 4 pairs' transposed weights [d=128, 4*128] moving (N=512, the fp32 moving
limit), producing PSUM [b=128, 4*128] directly in the natural output layout.
VectorE applies the v_j gate straight out of PSUM into an SBUF staging tile
(fused PSUM-read + multiply + SBUF-write), and the staging tile is DMA'd out
with >= 512B-contiguous rows.

DMA engine assignment: all input loads go through SWDGE (GpSimd) so they can
never queue behind backpressured output stores; output stores alternate
between the two HWDGE rings (SP and ACT).  Per core traffic: 44 MB in (W
28.5 + two embedding layouts 15.3) + 114 MB out.  Measured on HW (marginal
time of an in-NEFF repeat loop): ~0.50 ms/call -- simultaneously at the PE
fp32 limit (480 self-loading fp32 matmuls) and the HBM limit.  Output is
bit-identical to an fp32 jax reference on-device and ~2e-7 Frobenius relative
error vs CPU BLAS.
"""

import os
import sys

import numpy as np

for _p in ("/opt/trn_rl_repo", "/root/.axon_site/_ro/trn_rl_repo"):
    if os.path.isdir(_p) and _p not in sys.path:
        sys.path.append(_p)

import concourse.bacc as bacc
import concourse.tile as tile
from concourse import mybir
from concourse.bass_utils import run_bass_kernel_spmd

B, F, D = 4096, 30, 128
P = F * (F - 1) // 2  # 435
NCORES = 8
BSH = B // NCORES  # 512 batches per core
NCHUNK = BSH // 128  # 4 batch chunks of 128
GROUP = 4  # pairs per matmul -> moving dim 512 (fp32 max)
FD = F * D  # 3840
PD = P * D  # 55680

TRACE = False
last_results = None  # BassKernelResults of the most recent kernel() call

_cache = {}


def _build(niter=1, mode="load", ftl_bufs=3, mm_dt=None, ps_bufs=None, ablate=None, stg_bufs=4, w_bufs=3, wide=0):
    nc = bacc.Bacc("TRN2", target_bir_lowering=False, debug=False, num_devices=NCORES)
    femb_n = nc.declare_dram_parameter("femb_n", [BSH, FD], mybir.dt.float32, isOutput=False)
    if mode == "load":
        femb_t = nc.declare_dram_parameter("femb_t", [FD, BSH], mybir.dt.float32, isOutput=False)
    w_t = nc.declare_dram_parameter("w_t", [D, PD], mybir.dt.float32, isOutput=False)
    if mode != "load":
        eye = nc.declare_dram_parameter("eye", [D, D], mybir.dt.float32, isOutput=False)
    out = nc.declare_dram_parameter("out", [BSH, PD], mybir.dt.float32, isOutput=True)

    import contextlib

    with tile.TileContext(nc) as tc:
        with (
            tc.tile_pool(name="eye", bufs=1) as eye_pool,
            tc.tile_pool(name="fn", bufs=1) as fn_pool,
            tc.tile_pool(name="ftl", bufs=ftl_bufs) as ftl_pool,
            tc.tile_pool(name="w", bufs=w_bufs) as w_pool,
            tc.tile_pool(name="stg", bufs=stg_bufs) as stg_pool,
            tc.tile_pool(name="ps", bufs=ps_bufs or 6, space="PSUM") as ps_pool,
            tc.tile_pool(name="tr", bufs=2, space="PSUM") as tr_pool,
            tc.For_i(
                0,
                niter,
                1,
                hint_engines=(
                    mybir.EngineType.PE,
                    mybir.EngineType.DVE,
                    mybir.EngineType.Activation,
                    mybir.EngineType.SP,
                ),
            )
            if niter > 1
            else contextlib.nullcontext(),
        ):
            if mode != "load":
                eye_tile = eye_pool.tile([D, D], mybir.dt.float32)
                nc.gpsimd.dma_start(eye_tile[:], eye[:])
            # whole femb shard, natural layout: partition=b (within chunk),
            # free=(field, emb); one tile per batch chunk so consumers only
            # wait on the chunk they need.
            fn_tiles = []
            for c in range(NCHUNK):
                fnt = fn_pool.tile([128, FD], mybir.dt.float32, tag=f"fn{c}")
                nc.gpsimd.dma_start(fnt[:], femb_n[c * 128 : (c + 1) * 128, :])
                fn_tiles.append(fnt)

            p0 = 0
            for i in range(F - 1):
                s = F - 1 - i  # pairs in this i-block: (i, i+1) .. (i, F-1)
                # Build v_i in [d, b] layout on-chip: PE transpose-mode
                # (exact data movement) + ScalarE copy out of PSUM.
                ftl_tile = ftl_pool.tile([128, BSH], mybir.dt.float32, tag="ftl")
                if mode == "load":
                    nc.gpsimd.dma_start(ftl_tile[:], femb_t[i * D : (i + 1) * D, :])
                else:
                    for c in range(NCHUNK):
                        trp = tr_pool.tile([128, 128], mybir.dt.float32, tag="tr")
                        nc.tensor.transpose(
                            trp[:], fn_tiles[c][:, i * D : (i + 1) * D], eye_tile[:]
                        )
                        nc.vector.tensor_copy(
                            ftl_tile[:, c * 128 : (c + 1) * 128], trp[:]
                        )

                w_tile = w_pool.tile([128, s * D], mybir.dt.float32, tag="w")
                nc.gpsimd.dma_start(w_tile[:], w_t[:, p0 * D : (p0 + s) * D])

                if wide:
                    # One output DMA per pair-window covering all 4 batch
                    # chunks (bigger transfers, better HBM write efficiency).
                    out3 = out.reshape([NCHUNK, 128, PD])
                    for w0 in range(0, s, wide):
                        nw = min(wide, s - w0)
                        stg_tile = stg_pool.tile(
                            [128, NCHUNK * wide * D], mybir.dt.float32, tag="stg"
                        )
                        for c in range(NCHUNK):
                            for q in range(w0, w0 + nw, GROUP):
                                ng = min(GROUP, w0 + nw - q)
                                ps = ps_pool.tile(
                                    [128, GROUP * D], mybir.dt.float32, tag="ps"
                                )
                                nc.tensor.matmul(
                                    ps[:, : ng * D],
                                    ftl_tile[:, c * 128 : (c + 1) * 128],
                                    w_tile[:, q * D : (q + ng) * D],
                                    start=True,
                                    stop=True,
                                )
                                j0 = i + 1 + q
                                off = (c * nw + (q - w0)) * D
                                nc.vector.tensor_mul(
                                    stg_tile[:, off : off + ng * D],
                                    ps[:, : ng * D],
                                    fn_tiles[c][:, j0 * D : (j0 + ng) * D],
                                )
                        out_eng = nc.sync if (i + w0) % 2 == 0 else nc.scalar
                        out_eng.dma_start(
                            out3[:, :, (p0 + w0) * D : (p0 + w0 + nw) * D]
                            .transpose((1, 0, 2)),
                            stg_tile[:, : NCHUNK * nw * D],
                        )
                    p0 += s
                    continue
                for c in range(NCHUNK):
                    stg_tile = stg_pool.tile([128, s * D], mybir.dt.float32, tag="stg")
                    if ablate == "nocompute":
                        nc.vector.tensor_scalar_mul(
                            stg_tile[:, 0:4], stg_tile[:, 0:4], 0.0
                        )
                    for q in range(0, s, GROUP) if ablate != "nocompute" else []:
                        ng = min(GROUP, s - q)
                        ps = ps_pool.tile([128, GROUP * D], mybir.dt.float32, tag="ps")
                        lhsT = ftl_tile[:, c * 128 : (c + 1) * 128]  # [K=d, M=b]
                        rhs = w_tile[:, q * D : (q + ng) * D]  # [K=d, N=pairs*e]
                        if mm_dt is not None:
                            lhsT = lhsT.bitcast(mm_dt)
                            rhs = rhs.bitcast(mm_dt)
                        nc.tensor.matmul(ps[:, : ng * D], lhsT, rhs, start=True, stop=True)
                        j0 = i + 1 + q
                        nc.vector.tensor_mul(
                            stg_tile[:, q * D : (q + ng) * D],
                            ps[:, : ng * D],
                            fn_tiles[c][:, j0 * D : (j0 + ng) * D],
                        )
                    if ablate != "noout":
                        out_eng = nc.sync if (i * NCHUNK + c) % 2 == 0 else nc.scalar
                        out_eng.dma_start(
                            out[c * 128 : (c + 1) * 128, p0 * D : (p0 + s) * D],
                            stg_tile[:],
                        )
                p0 += s

    nc.compile()
    return nc


def _input_names(nc):
    names = set()
    for alloc in nc.m.functions[0].allocations:
        if isinstance(alloc, mybir.MemoryLocationSet) and alloc.kind == "ExternalInput":
            names.add(alloc.memorylocations[0].name)
    return names


def kernel(feature_emb, W):
    global last_results
    femb = np.ascontiguousarray(feature_emb, dtype=np.float32)
    Wc = np.asarray(W, dtype=np.float32)
    assert femb.shape == (B, F, D) and Wc.shape == (P, D, D)

    if "nc" not in _cache:
        _cache["nc"] = _build()
    nc = _cache["nc"]

    # w_t[d, p*D + e] = W[p, e, d]
    w_t = np.ascontiguousarray(Wc.transpose(2, 0, 1)).reshape(D, PD)
    eye = np.eye(D, dtype=np.float32)
    ft_all = femb.transpose(1, 2, 0)  # [F, D, B] view
    in_maps = []
    for co in range(NCORES):
        sl = slice(co * BSH, (co + 1) * BSH)
        in_maps.append(
            {
                "femb_n": femb[sl].reshape(BSH, FD),
                "femb_t": np.ascontiguousarray(ft_all[:, :, sl]).reshape(FD, BSH),
                "w_t": w_t,
                "eye": eye,
            }
        )
    in_maps = [
        {k: v for k, v in m.items() if k in _input_names(nc)} for m in in_maps
    ]

    res = run_bass_kernel_spmd(nc, in_maps, list(range(NCORES)), trace=TRACE)
    last_results = res

    out = np.empty((B, P, D), dtype=np.float32)
    for co in range(NCORES):
        out[co * BSH : (co + 1) * BSH] = res.results[co]["out"].reshape(BSH, P, D)
    return out


# ---------------------------------------------------------------------------
# Timing support (used by test.py; not needed for grading correctness).
# The local axon build has no NTFF profile hook, so HW time is measured as the
# marginal wall-clock of an in-NEFF repeat loop with device-resident inputs:
# t(niter=N) - t(niter=1) cancels all host/tunnel/launch constants.
# ---------------------------------------------------------------------------


def _make_runner(nc, n_cores=NCORES):
    import jax
    import jax.numpy as jnp
    from jax.sharding import Mesh, NamedSharding, PartitionSpec
    from jax.experimental.shard_map import shard_map

    from concourse import bass2jax

    bass2jax.install_neuronx_cc_hook()
    partition_name = nc.partition_id_tensor.name if nc.partition_id_tensor else None
    in_names, out_names, out_avals = [], [], []
    for alloc in nc.m.functions[0].allocations:
        if not isinstance(alloc, mybir.MemoryLocationSet):
            continue
        name = alloc.memorylocations[0].name
        if alloc.kind == "ExternalInput":
            if name != partition_name:
                in_names.append(name)
        elif alloc.kind == "ExternalOutput":
            out_names.append(name)
            out_avals.append(
                jax.core.ShapedArray(tuple(alloc.tensor_shape), mybir.dt.np(alloc.dtype))
            )
    n_params, n_outs = len(in_names), len(out_names)
    all_names = in_names + out_names + ([partition_name] if partition_name else [])

    def _body(*args):
        operands = list(args)
        if partition_name is not None:
            operands.append(bass2jax.partition_id_tensor())
        return tuple(
            bass2jax._bass_exec_p.bind(
                *operands,
                out_avals=tuple(out_avals),
                in_names=tuple(all_names),
                out_names=tuple(out_names),
                lowering_input_output_aliases=(),
                sim_require_finite=True,
                sim_require_nnan=True,
                nc=nc,
            )
        )

    mesh = Mesh(np.asarray(jax.devices()[:n_cores]), ("core",))
    spec = PartitionSpec("core")
    sharded = jax.jit(
        shard_map(
            _body,
            mesh=mesh,
            in_specs=(spec,) * (n_params + n_outs),
            out_specs=(spec,) * n_outs,
            check_rep=False,
        ),
        donate_argnums=tuple(range(n_params, n_params + n_outs)),
        keep_unused=True,
    )
    sharding = NamedSharding(mesh, spec)
    zeros_fn = jax.jit(
        lambda: tuple(
            jnp.zeros((n_cores * a.shape[0], *a.shape[1:]), a.dtype) for a in out_avals
        ),
        out_shardings=(sharding,) * n_outs,
    )
    return sharded, zeros_fn, in_names, sharding


def _bench_once(niter, in_maps, reps=4):
    import time

    import jax

    nc = _build(niter=niter)
    sharded, zeros_fn, in_names, sharding = _make_runner(nc)
    dev_in = [
        jax.device_put(np.concatenate([m[n] for m in in_maps], axis=0), sharding)
        for n in in_names
    ]
    for a in dev_in:
        a.block_until_ready()
    times = []
    for _ in range(reps):
        zeros = zeros_fn()
        for z in zeros:
            z.block_until_ready()
        t0 = time.time()
        outs = sharded(*dev_in, *zeros)
        for o in outs:
            o.block_until_ready()
        times.append(time.time() - t0)
    return min(times)


def measure_hw_time_ns(feature_emb, W, niter=101, reps=5):
    """Marginal per-iteration HW time of the kernel NEFF, in ns."""
    femb = np.ascontiguousarray(feature_emb, dtype=np.float32)
    Wc = np.asarray(W, dtype=np.float32)
    w_t = np.ascontiguousarray(Wc.transpose(2, 0, 1)).reshape(D, PD)
    eye = np.eye(D, dtype=np.float32)
    ft_all = femb.transpose(1, 2, 0)
    in_maps = []
    for co in range(NCORES):
        sl = slice(co * BSH, (co + 1) * BSH)
        in_maps.append(
            {
                "femb_n": femb[sl].reshape(BSH, FD),
                "femb_t": np.ascontiguousarray(ft_all[:, :, sl]).reshape(FD, BSH),
                "w_t": w_t,
                "eye": eye,
            }
        )
    t1 = _bench_once(1, in_maps, reps)
    tn = _bench_once(niter, in_maps, reps)
    return (tn - t1) / (niter - 1) * 1e9, t1, tn


# revision 19
# speedup vs baseline: 1.0334x; 1.0334x over previous
"""Bilinear interaction layer (pairwise per-field Linear + gate) on 8 trn2 cores.

out[b, p, :] = (femb[b, i_p] @ W[p].T) * femb[b, j_p]   for the P=C(F,2) field
pairs (i_p, j_p) in itertools.combinations order.  B=4096, F=30, D=128, P=435.

Sharding: data-parallel over batch (4096 -> 512 per core), W replicated.

Per core, pairs are processed in "i-blocks" (the (29-i) pairs sharing first
field i, consecutive in p).  For each i-block and each 128-row batch chunk,
TensorE runs fp32 matmuls with the v_i chunk [d=128, b=128] stationary and up
to 4 pairs' transposed weights [d=128, 4*128] moving (N=512, the fp32 moving
limit), producing PSUM [b=128, 4*128] directly in the natural output layout.
VectorE applies the v_j gate straight out of PSUM into an SBUF staging tile
(fused PSUM-read + multiply + SBUF-write), and the staging tile is DMA'd out
with >= 512B-contiguous rows.

DMA engine assignment: all input loads go through SWDGE (GpSimd) so they can
never queue behind backpressured output stores; output stores alternate
between the two HWDGE rings (SP and ACT).  Per core traffic: 44 MB in (W
28.5 + two embedding layouts 15.3) + 114 MB out.  Measured on HW (marginal
time of an in-NEFF repeat loop): ~0.50 ms/call -- simultaneously at the PE
fp32 limit (480 self-loading fp32 matmuls) and the HBM limit.  Output is
bit-identical to an fp32 jax reference on-device and ~2e-7 Frobenius relative
error vs CPU BLAS.
"""

import os
import sys

import numpy as np

for _p in ("/opt/trn_rl_repo", "/root/.axon_site/_ro/trn_rl_repo"):
    if os.path.isdir(_p) and _p not in sys.path:
        sys.path.append(_p)

import concourse.bacc as bacc
import concourse.tile as tile
from concourse import mybir
from concourse.bass_utils import run_bass_kernel_spmd

B, F, D = 4096, 30, 128
P = F * (F - 1) // 2  # 435
NCORES = 8
BSH = B // NCORES  # 512 batches per core
NCHUNK = BSH // 128  # 4 batch chunks of 128
GROUP = 4  # pairs per matmul -> moving dim 512 (fp32 max)
FD = F * D  # 3840
PD = P * D  # 55680

TRACE = False
last_results = None  # BassKernelResults of the most recent kernel() call

_cache = {}


def _build(niter=1, mode="load", ftl_bufs=3, mm_dt=None, ps_bufs=None, ablate=None, stg_bufs=4, w_bufs=3, wide=0, out_rings=2):
    nc = bacc.Bacc("TRN2", target_bir_lowering=False, debug=False, num_devices=NCORES)
    femb_n = nc.declare_dram_parameter("femb_n", [BSH, FD], mybir.dt.float32, isOutput=False)
    if mode == "load":
        femb_t = nc.declare_dram_parameter("femb_t", [FD, BSH], mybir.dt.float32, isOutput=False)
    w_t = nc.declare_dram_parameter("w_t", [D, PD], mybir.dt.float32, isOutput=False)
    if mode != "load":
        eye = nc.declare_dram_parameter("eye", [D, D], mybir.dt.float32, isOutput=False)
    out = nc.declare_dram_parameter("out", [BSH, PD], mybir.dt.float32, isOutput=True)

    import contextlib

    with tile.TileContext(nc) as tc:
        with (
            tc.tile_pool(name="eye", bufs=1) as eye_pool,
            tc.tile_pool(name="fn", bufs=1) as fn_pool,
            tc.tile_pool(name="ftl", bufs=ftl_bufs) as ftl_pool,
            tc.tile_pool(name="w", bufs=w_bufs) as w_pool,
            tc.tile_pool(name="stg", bufs=stg_bufs) as stg_pool,
            tc.tile_pool(name="ps", bufs=ps_bufs or 6, space="PSUM") as ps_pool,
            tc.tile_pool(name="tr", bufs=2, space="PSUM") as tr_pool,
            tc.For_i(
                0,
                niter,
                1,
                hint_engines=(
                    mybir.EngineType.PE,
                    mybir.EngineType.DVE,
                    mybir.EngineType.Activation,
                    mybir.EngineType.SP,
                ),
            )
            if niter > 1
            else contextlib.nullcontext(),
        ):
            if mode != "load":
                eye_tile = eye_pool.tile([D, D], mybir.dt.float32)
                nc.gpsimd.dma_start(eye_tile[:], eye[:])
            # whole femb shard, natural layout: partition=b (within chunk),
            # free=(field, emb); one tile per batch chunk so consumers only
            # wait on the chunk they need.
            fn_tiles = []
            for c in range(NCHUNK):
                fnt = fn_pool.tile([128, FD], mybir.dt.float32, tag=f"fn{c}")
                nc.gpsimd.dma_start(fnt[:], femb_n[c * 128 : (c + 1) * 128, :])
                fn_tiles.append(fnt)

            p0 = 0
            for i in range(F - 1):
                s = F - 1 - i  # pairs in this i-block: (i, i+1) .. (i, F-1)
                # Build v_i in [d, b] layout on-chip: PE transpose-mode
                # (exact data movement) + ScalarE copy out of PSUM.
                ftl_tile = ftl_pool.tile([128, BSH], mybir.dt.float32, tag="ftl")
                if mode == "load":
                    nc.gpsimd.dma_start(ftl_tile[:], femb_t[i * D : (i + 1) * D, :])
                else:
                    for c in range(NCHUNK):
                        trp = tr_pool.tile([128, 128], mybir.dt.float32, tag="tr")
                        nc.tensor.transpose(
                            trp[:], fn_tiles[c][:, i * D : (i + 1) * D], eye_tile[:]
                        )
                        nc.vector.tensor_copy(
                            ftl_tile[:, c * 128 : (c + 1) * 128], trp[:]
                        )

                w_tile = w_pool.tile([128, s * D], mybir.dt.float32, tag="w")
                nc.gpsimd.dma_start(w_tile[:], w_t[:, p0 * D : (p0 + s) * D])

                if wide:
                    # One output DMA per pair-window covering all 4 batch
                    # chunks (bigger transfers, better HBM write efficiency).
                    out3 = out.reshape([NCHUNK, 128, PD])
                    for w0 in range(0, s, wide):
                        nw = min(wide, s - w0)
                        stg_tile = stg_pool.tile(
                            [128, NCHUNK * wide * D], mybir.dt.float32, tag="stg"
                        )
                        for c in range(NCHUNK):
                            for q in range(w0, w0 + nw, GROUP):
                                ng = min(GROUP, w0 + nw - q)
                                ps = ps_pool.tile(
                                    [128, GROUP * D], mybir.dt.float32, tag="ps"
                                )
                                nc.tensor.matmul(
                                    ps[:, : ng * D],
                                    ftl_tile[:, c * 128 : (c + 1) * 128],
                                    w_tile[:, q * D : (q + ng) * D],
                                    start=True,
                                    stop=True,
                                )
                                j0 = i + 1 + q
                                off = (c * nw + (q - w0)) * D
                                nc.vector.tensor_mul(
                                    stg_tile[:, off : off + ng * D],
                                    ps[:, : ng * D],
                                    fn_tiles[c][:, j0 * D : (j0 + ng) * D],
                                )
                        out_eng = nc.sync if (i + w0) % 2 == 0 else nc.scalar
                        out_eng.dma_start(
                            out3[:, :, (p0 + w0) * D : (p0 + w0 + nw) * D]
                            .transpose((1, 0, 2)),
                            stg_tile[:, : NCHUNK * nw * D],
                        )
                    p0 += s
                    continue
                for c in range(NCHUNK):
                    stg_tile = stg_pool.tile([128, s * D], mybir.dt.float32, tag="stg")
                    if ablate == "nocompute":
                        nc.vector.tensor_scalar_mul(
                            stg_tile[:, 0:4], stg_tile[:, 0:4], 0.0
                        )
                    for q in range(0, s, GROUP) if ablate != "nocompute" else []:
                        ng = min(GROUP, s - q)
                        ps = ps_pool.tile([128, GROUP * D], mybir.dt.float32, tag="ps")
                        lhsT = ftl_tile[:, c * 128 : (c + 1) * 128]  # [K=d, M=b]
                        rhs = w_tile[:, q * D : (q + ng) * D]  # [K=d, N=pairs*e]
                        if mm_dt is not None:
                            lhsT = lhsT.bitcast(mm_dt)
                            rhs = rhs.bitcast(mm_dt)
                        nc.tensor.matmul(ps[:, : ng * D], lhsT, rhs, start=True, stop=True)
                        j0 = i + 1 + q
                        nc.vector.tensor_mul(
                            stg_tile[:, q * D : (q + ng) * D],
                            ps[:, : ng * D],
                            fn_tiles[c][:, j0 * D : (j0 + ng) * D],
                        )
                    if ablate != "noout":
                        rings = [nc.sync, nc.scalar, nc.gpsimd][:out_rings]
                        out_eng = rings[(i * NCHUNK + c) % len(rings)]
                        out_eng.dma_start(
                            out[c * 128 : (c + 1) * 128, p0 * D : (p0 + s) * D],
                            stg_tile[:],
                        )
                p0 += s

    nc.compile()
    return nc


def _input_names(nc):
    names = set()
    for alloc in nc.m.functions[0].allocations:
        if isinstance(alloc, mybir.MemoryLocationSet) and alloc.kind == "ExternalInput":
            names.add(alloc.memorylocations[0].name)
    return names


def kernel(feature_emb, W):
    global last_results
    femb = np.ascontiguousarray(feature_emb, dtype=np.float32)
    Wc = np.asarray(W, dtype=np.float32)
    assert femb.shape == (B, F, D) and Wc.shape == (P, D, D)

    if "nc" not in _cache:
        _cache["nc"] = _build()
    nc = _cache["nc"]

    # w_t[d, p*D + e] = W[p, e, d]
    w_t = np.ascontiguousarray(Wc.transpose(2, 0, 1)).reshape(D, PD)
    eye = np.eye(D, dtype=np.float32)
    ft_all = femb.transpose(1, 2, 0)  # [F, D, B] view
    in_maps = []
    for co in range(NCORES):
        sl = slice(co * BSH, (co + 1) * BSH)
        in_maps.append(
            {
                "femb_n": femb[sl].reshape(BSH, FD),
                "femb_t": np.ascontiguousarray(ft_all[:, :, sl]).reshape(FD, BSH),
                "w_t": w_t,
                "eye": eye,
            }
        )
    in_maps = [
        {k: v for k, v in m.items() if k in _input_names(nc)} for m in in_maps
    ]

    res = run_bass_kernel_spmd(nc, in_maps, list(range(NCORES)), trace=TRACE)
    last_results = res

    out = np.empty((B, P, D), dtype=np.float32)
    for co in range(NCORES):
        out[co * BSH : (co + 1) * BSH] = res.results[co]["out"].reshape(BSH, P, D)
    return out


# ---------------------------------------------------------------------------
# Timing support (used by test.py; not needed for grading correctness).
# The local axon build has no NTFF profile hook, so HW time is measured as the
# marginal wall-clock of an in-NEFF repeat loop with device-resident inputs:
# t(niter=N) - t(niter=1) cancels all host/tunnel/launch constants.
# ---------------------------------------------------------------------------


def _make_runner(nc, n_cores=NCORES):
    import jax
    import jax.numpy as jnp
    from jax.sharding import Mesh, NamedSharding, PartitionSpec
    from jax.experimental.shard_map import shard_map

    from concourse import bass2jax

    bass2jax.install_neuronx_cc_hook()
    partition_name = nc.partition_id_tensor.name if nc.partition_id_tensor else None
    in_names, out_names, out_avals = [], [], []
    for alloc in nc.m.functions[0].allocations:
        if not isinstance(alloc, mybir.MemoryLocationSet):
            continue
        name = alloc.memorylocations[0].name
        if alloc.kind == "ExternalInput":
            if name != partition_name:
                in_names.append(name)
        elif alloc.kind == "ExternalOutput":
            out_names.append(name)
            out_avals.append(
                jax.core.ShapedArray(tuple(alloc.tensor_shape), mybir.dt.np(alloc.dtype))
            )
    n_params, n_outs = len(in_names), len(out_names)
    all_names = in_names + out_names + ([partition_name] if partition_name else [])

    def _body(*args):
        operands = list(args)
        if partition_name is not None:
            operands.append(bass2jax.partition_id_tensor())
        return tuple(
            bass2jax._bass_exec_p.bind(
                *operands,
                out_avals=tuple(out_avals),
                in_names=tuple(all_names),
                out_names=tuple(out_names),
                lowering_input_output_aliases=(),
                sim_require_finite=True,
                sim_require_nnan=True,
                nc=nc,
            )
        )

    mesh = Mesh(np.asarray(jax.devices()[:n_cores]), ("core",))
    spec = PartitionSpec("core")
    sharded = jax.jit(
        shard_map(
            _body,
            mesh=mesh,
            in_specs=(spec,) * (n_params + n_outs),
            out_specs=(spec,) * n_outs,
            check_rep=False,
        ),
        donate_argnums=tuple(range(n_params, n_params + n_outs)),
        keep_unused=True,
    )
    sharding = NamedSharding(mesh, spec)
    zeros_fn = jax.jit(
        lambda: tuple(
            jnp.zeros((n_cores * a.shape[0], *a.shape[1:]), a.dtype) for a in out_avals
        ),
        out_shardings=(sharding,) * n_outs,
    )
    return sharded, zeros_fn, in_names, sharding


def _bench_once(niter, in_maps, reps=4):
    import time

    import jax

    nc = _build(niter=niter)
    sharded, zeros_fn, in_names, sharding = _make_runner(nc)
    dev_in = [
        jax.device_put(np.concatenate([m[n] for m in in_maps], axis=0), sharding)
        for n in in_names
    ]
    for a in dev_in:
        a.block_until_ready()
    times = []
    for _ in range(reps):
        zeros = zeros_fn()
        for z in zeros:
            z.block_until_ready()
        t0 = time.time()
        outs = sharded(*dev_in, *zeros)
        for o in outs:
            o.block_until_ready()
        times.append(time.time() - t0)
    return min(times)


def measure_hw_time_ns(feature_emb, W, niter=101, reps=5):
    """Marginal per-iteration HW time of the kernel NEFF, in ns."""
    femb = np.ascontiguousarray(feature_emb, dtype=np.float32)
    Wc = np.asarray(W, dtype=np.float32)
    w_t = np.ascontiguousarray(Wc.transpose(2, 0, 1)).reshape(D, PD)
    eye = np.eye(D, dtype=np.float32)
    ft_all = femb.transpose(1, 2, 0)
    in_maps = []
    for co in range(NCORES):
        sl = slice(co * BSH, (co + 1) * BSH)
        in_maps.append(
            {
                "femb_n": femb[sl].reshape(BSH, FD),
                "femb_t": np.ascontiguousarray(ft_all[:, :, sl]).reshape(FD, BSH),
                "w_t": w_t,
                "eye": eye,
            }
        )
    t1 = _bench_once(1, in_maps, reps)
    tn = _bench_once(niter, in_maps, reps)
    return (tn - t1) / (niter - 1) * 1e9, t1, tn
